# revision 29
# baseline (speedup 1.0000x reference)
"""Trainium2 Bass kernel for nn_MixtureOfRoutingAttention.

Strategy: data-parallel over B=8 (one sample per NeuronCore). Top-1 routing
runs on host (it only decides dispatch); selected expert weights are gathered
per sample, LayerNorm affines folded into them, pre-scaled by 32 and shipped
as fp8-e4m3. All projection / PV / MLP-up GEMMs run in fp8 DoubleRow mode
(two 128-deep contraction tiles per instruction); scores and MLP-down stay
bf16. PSUM accumulates fp32 throughout.

Key precision bookkeeping: weights carry x32, activations x1, so projection
psums are x32 (descaled at eviction via the ACT scale operand). V-values and
the attention numerators carry x32 all the way into the out-projection psum
(x1024), descaled once at its eviction. The softmax denominator comes from a
trailing ones-column in V (unscaled), so deferred normalization divides out
correctly. LayerNorm affine folding makes the spatial and temporal branches
share one normalized activation tensor xc (the temporal residual re-applies
gamma_t via one fused DVE op per block).
"""

import math
import os
from contextlib import ExitStack

import numpy as np
import ml_dtypes

import concourse.bass as bass
import concourse.bacc as bacc
import concourse.tile as tile
import concourse.mybir as mybir
from concourse import bass_utils

dt = mybir.dt
AF = mybir.ActivationFunctionType
ALU = mybir.AluOpType
PM = mybir.MatmulPerfMode

P = 128
T = 512
D = 768
H = 8
HD = 96
DFF = 3072
NCORES = 8
ND = D // P  # 6
NP = ND // 2  # 3 feature-block pairs
NT = T // P  # 4
NF = DFF // P  # 24
FCC = 4  # mlp fc-chunk (DMA granularity)
SCALE = 1.0 / math.sqrt(HD)
EPS = 1e-5
WS = 32.0  # fp8 weight pre-scale
RS = 1.0 / WS
F32 = dt.float32
BF = dt.bfloat16
F8 = dt.float8e4
NPBF = ml_dtypes.bfloat16
NPF8 = ml_dtypes.float8_e4m3

PHASE_MARKS = []


def build(repeat: int = 1):
    PHASE_MARKS.clear()
    nc = bacc.Bacc(
        "TRN2",
        target_bir_lowering=False,
        debug=False,
        enable_asserts=False,
        num_devices=NCORES,
    )

    def din(name, shape, dtype=F8):
        return nc.dram_tensor(name, shape, dtype, kind="ExternalInput").ap()

    xT_d = din("xT", [D, T], BF)
    mask_d = din("mask", [P, P], F32)
    gt_d = din("g_t", [D], F32)

    wqk_d = {b: din(f"{b}Wqk", [P, 2 * H * ND * HD]) for b in ("sp", "tp", "cx")}
    wv_d = {b: din(f"{b}Wv", [P, ND * D]) for b in ("sp", "tp", "cx")}
    wo_d = {b: din(f"{b}Wo", [HD, ND * H * P]) for b in ("sp", "tp", "cx")}
    bq_d = {b: din(f"{b}Bq", [D], F32) for b in ("sp", "tp", "cx")}
    bk_d = {b: din(f"{b}Bk", [D], F32) for b in ("sp", "tp", "cx")}
    bv_d = {b: din(f"{b}Bv", [D], F32) for b in ("sp", "tp", "cx")}
    bo_d = {b: din(f"{b}Bo", [D], F32) for b in ("sp", "tp", "cx")}

    mW1_d = din("mW1", [P, NF * ND * P])
    mB1_d = din("mB1", [DFF], F32)
    mW2_d = din("mW2", [P, NF * D], BF)
    mB2_d = din("mB2", [D], F32)

    outT_d = nc.dram_tensor("outT", [D, T], BF, kind="ExternalOutput").ap()

    with tile.TileContext(nc) as tc, ExitStack() as ctx:
        ctx.enter_context(
            nc.allow_low_precision(reason="fp8 matmul operands, fp32 accumulation")
        )
        const = ctx.enter_context(tc.tile_pool(name="const", bufs=1))
        big = ctx.enter_context(tc.tile_pool(name="big", bufs=1))
        wqkp = ctx.enter_context(tc.tile_pool(name="wqkp", bufs=3))
        wvp = ctx.enter_context(tc.tile_pool(name="wvp", bufs=3))
        wop = ctx.enter_context(tc.tile_pool(name="wop", bufs=3))
        w1p = ctx.enter_context(tc.tile_pool(name="w1p", bufs=2))
        w2p = ctx.enter_context(tc.tile_pool(name="w2p", bufs=2))
        tmp = ctx.enter_context(tc.tile_pool(name="tmp", bufs=4))
        qk = ctx.enter_context(tc.tile_pool(name="qk", bufs=4))
        ex = ctx.enter_context(tc.tile_pool(name="ex", bufs=4))
        rows = ctx.enter_context(tc.tile_pool(name="rows", bufs=6))
        ps = ctx.enter_context(tc.tile_pool(name="ps", bufs=8, space="PSUM"))

        def ppt(nm):  # [P, 2, T] fp32 psum pair (2 banks), ring of 3
            return ps.tile([P, 2, T], F32, name=nm, tag="pp", bufs=3)

        def pst(nm):  # [P, T] fp32 psum single, ring of 2
            return ps.tile([P, T], F32, name=nm, tag="ps", bufs=2)

        # ---- constants / params (loaded once) ----
        ones_bf = const.tile([P, 1], BF, name="ones_bf", tag="ones")
        nc.vector.memset(ones_bf, 1.0)
        maskc = const.tile([P, P], F32, name="maskc", tag="mask")
        nc.sync.dma_start(out=maskc, in_=mask_d)
        epsc = const.tile([P, 1], F32, name="epsc", tag="eps")
        nc.vector.memset(epsc, EPS)
        gt_sb = const.tile([P, ND], F32, name="gt_sb", tag="gt")
        nc.sync.dma_start(out=gt_sb, in_=gt_d.rearrange("(a p) -> p a", p=P))

        bq96, bk96, vbias, bo_sb = {}, {}, {}, {}
        for b in ("sp", "tp", "cx"):
            t = const.tile([HD, H], F32, name=f"bq96_{b}", tag=f"bq96_{b}")
            nc.sync.dma_start(out=t, in_=bq_d[b].rearrange("(h k) -> k h", k=HD))
            bq96[b] = t
            t = const.tile([HD, H], F32, name=f"bk96_{b}", tag=f"bk96_{b}")
            nc.sync.dma_start(out=t, in_=bk_d[b].rearrange("(h k) -> k h", k=HD))
            bk96[b] = t
            t = const.tile([P, D], F32, name=f"vb_{b}", tag=f"vb_{b}")
            nc.gpsimd.dma_start(
                out=t,
                in_=bass.AP(
                    tensor=bv_d[b].tensor,
                    offset=bv_d[b].offset,
                    ap=[[0, P], list(bv_d[b].ap[0])],
                ),
            )
            vbias[b] = t
            t = const.tile([P, ND], F32, name=f"bo_{b}", tag=f"bo_{b}")
            nc.sync.dma_start(out=t, in_=bo_d[b].rearrange("(a p) -> p a", p=P))
            bo_sb[b] = t

        mB1_sb = const.tile([P, NF], F32, name="mB1_sb", tag="mB1")
        nc.sync.dma_start(out=mB1_sb, in_=mB1_d.rearrange("(a p) -> p a", p=P))
        mB2_sb = const.tile([P, ND], F32, name="mB2_sb", tag="mB2")
        nc.sync.dma_start(out=mB2_sb, in_=mB2_d.rearrange("(a p) -> p a", p=P))

        # ---- helpers ----

        def ln_feed(ps_m, ps_s, src_pairs, i):
            """Accumulate sum(x) / sum(x^2) column sums for pair i."""
            for j in range(2):
                sq = tmp.tile([P, T], BF, name="sq", tag="tmp")
                nc.gpsimd.tensor_mul(sq, src_pairs[i][:, j, :], src_pairs[i][:, j, :])
                nc.tensor.matmul(
                    ps_m[0:1, :], ones_bf, src_pairs[i][:, j, :],
                    start=(i == 0 and j == 0), stop=(i == NP - 1 and j == 1),
                )
                nc.tensor.matmul(
                    ps_s[0:1, :], ones_bf, sq,
                    start=(i == 0 and j == 0), stop=(i == NP - 1 and j == 1),
                )

        def ln_all(src_pairs, xc_tag):
            ps_m = pst("ps_m")
            ps_s = pst("ps_s")
            for i in range(NP):
                ln_feed(ps_m, ps_s, src_pairs, i)
            return ln_finish(ps_m, ps_s, src_pairs, xc_tag)

        def ln_finish(ps_m, ps_s, src_pairs, xc_tag):
            """Returns 3 x [P, 2, T] fp8 xc = (x - mean) * rstd."""
            mrow = rows.tile([1, T], F32, name="mrow", tag="rows")
            nc.vector.tensor_scalar_mul(mrow, ps_m[0:1, :], 1.0 / D)
            m2 = rows.tile([1, T], F32, name="m2", tag="rows")
            nc.vector.tensor_mul(m2, mrow, mrow)
            var = rows.tile([1, T], F32, name="var", tag="rows")
            nc.vector.scalar_tensor_tensor(
                var, ps_s[0:1, :], 1.0 / D, m2, ALU.mult, ALU.subtract
            )
            u = rows.tile([1, T], F32, name="u", tag="rows")
            nc.scalar.activation(u, var, AF.Ln, bias=epsc[0:1, :])
            rrow = rows.tile([1, T], BF, name="rrow", tag="rows")
            nc.scalar.activation(rrow, u, AF.Exp, scale=-0.5)
            mrow_bf = rows.tile([1, T], BF, name="mrow_bf", tag="rows")
            nc.vector.tensor_copy(mrow_bf, mrow)

            meanb = big.tile([P, T], BF, name="meanb", tag="meanb", bufs=2)
            nc.gpsimd.partition_broadcast(meanb, mrow_bf)
            rstdb = big.tile([P, T], BF, name="rstdb", tag="rstdb", bufs=2)
            nc.gpsimd.partition_broadcast(rstdb, rrow)

            xc = []
            for i in range(NP):
                t = big.tile([P, 2, T], F8, name=f"xc{i}", tag=f"{xc_tag}{i}", bufs=2)
                for j in range(2):
                    xm = tmp.tile([P, T], BF, name="xm", tag="tmp")
                    nc.vector.tensor_sub(xm, src_pairs[i][:, j, :], meanb)
                    nc.vector.tensor_mul(t[:, j, :], xm, rstdb)
                xc.append(t)
            return xc

        def load_wqk(b):
            t = wqkp.tile([P, 2, H, ND, HD], F8, name=f"wqk_{b}", tag="wqk")
            nc.sync.dma_start(
                out=t,
                in_=wqk_d[b].rearrange("p (q h a k) -> p q h a k", q=2, h=H, a=ND),
            )
            return t

        def load_wv(b):
            t = wvp.tile([P, ND, D], F8, name=f"wv_{b}", tag="wv")
            nc.sync.dma_start(out=t, in_=wv_d[b].rearrange("p (a e) -> p a e", a=ND))
            return t

        def load_wo(b):
            t = wop.tile([HD, ND, H, P], F8, name=f"wo_{b}", tag="wo")
            nc.sync.dma_start(
                out=t, in_=wo_d[b].rearrange("k (e h ec) -> k e h ec", e=ND, h=H)
            )
            return t

        def gemm_v(src_pairs, wv_sb, vbias_bc, Vt):
            """Vt: [P, NT, H, HD+2] fp8 (pad col keeps DR pair stride %16==0), V scaled x32, trailing ones column."""
            nc.vector.memset(Vt[:, :, :, HD], 1.0)
            for t in range(NT):
                pv = ppt("pv")
                for half in range(2):
                    for i in range(NP):
                        nc.tensor.matmul(
                            pv[:, half, 0:384],
                            src_pairs[i][:, :, t * P : (t + 1) * P],
                            wv_sb[:, 2 * i : 2 * i + 2, half * 384 : half * 384 + 384],
                            start=(i == 0), stop=(i == NP - 1),
                            perf_mode=PM.DoubleRow,
                        )
                nc.vector.tensor_add(
                    Vt[:, t, :, 0:HD].rearrange("p (a b) k -> p a b k", a=2),
                    pv[:, :, 0:384].rearrange("p a (b k) -> p a b k", k=HD),
                    vbias_bc.rearrange("p (a b k) -> p a b k", a=2, k=HD),
                )

        def attn_head_core(pqk, bq, bk, h, Vt, attnT8, causal, kq_act):
            """Evict q/k, scores, exp, PV, deferred-softmax normalize."""
            qh = qk.tile([P, T], BF, name="qh", tag="qk")
            kh = qk.tile([P, T], BF, name="kh", tag="qk")
            nc.scalar.activation(
                qh[0:HD, :], pqk[0:HD, 0, :], AF.Identity,
                bias=bq[:, h : h + 1], scale=RS,
            )
            if kq_act:
                nc.scalar.activation(
                    kh[0:HD, :], pqk[0:HD, 1, :], AF.Identity,
                    bias=bk[:, h : h + 1], scale=RS,
                )
            else:
                nc.vector.tensor_scalar(
                    kh[0:HD, :], pqk[0:HD, 1, :], RS, bk[:, h : h + 1],
                    ALU.mult, ALU.add,
                )
            ets = []
            for p in range(2):
                pS = ppt("pS")
                for j in range(2):
                    jc = 2 * p + j
                    i0 = jc * P if causal else 0
                    nc.tensor.matmul(
                        pS[:, j, i0:T],
                        kh[0:HD, jc * P : (jc + 1) * P],
                        qh[0:HD, i0:T],
                        start=True, stop=True,
                    )
                et = ex.tile([P, 2, T], F8, name="et", tag="ex")
                if causal:
                    # additive -1e9 mask on both diagonal blocks in one op
                    base = pS.offset + 2 * p * P
                    diag = bass.AP(
                        tensor=pS.tensor, offset=base,
                        ap=[list(pS.ap[0]), [T + P, 2], [1, P]],
                    )
                    mrep = bass.AP(
                        tensor=maskc.tensor, offset=maskc.offset,
                        ap=[list(maskc.ap[0]), [0, 2], [1, P]],
                    )
                    nc.vector.tensor_add(diag, diag, mrep)
                    for j in range(2):
                        jc = 2 * p + j
                        nc.scalar.activation(
                            et[:, j, jc * P : T], pS[:, j, jc * P : T],
                            AF.Exp, scale=SCALE,
                        )
                    # zero the causally-dead prefix of the second tile of the
                    # pair so the DoubleRow PV can span the pair's full range
                    z0 = 2 * p * P
                    nc.vector.memset(et[:, 1, z0 : z0 + P], 0.0)
                else:
                    nc.scalar.activation(et, pS, AF.Exp, scale=SCALE)
                ets.append(et)
            pa = pst("pa")
            for p in range(2):
                i0 = 2 * p * P if causal else 0
                nc.tensor.matmul(
                    pa[0 : HD + 1, i0:T],
                    Vt[:, 2 * p : 2 * p + 2, h, 0 : HD + 1],
                    ets[p][:, :, i0:T],
                    start=(p == 0), stop=(p == 1),
                    perf_mode=PM.DoubleRow,
                )
            srow = rows.tile([HD + 1, T], F32, name="srow", tag="srow", bufs=3)
            nc.vector.reciprocal(srow[HD : HD + 1, :], pa[HD : HD + 1, :])
            s = srow[HD : HD + 1, :]
            rbc = tmp.tile([HD, T], F32, name="rbc", tag="rbf", bufs=3)
            nc.gpsimd.dma_start(
                out=rbc,
                in_=bass.AP(
                    tensor=s.tensor, offset=s.offset,
                    ap=[list(s.ap[0]), [0, HD], list(s.ap[-1])],
                ),
            )
            nc.vector.tensor_mul(attnT8[0:HD, h, :], pa[0:HD, :], rbc)

        def attn_branch(srcq_pairs, srck_pairs, wqk_sb, bq, bk, Vt, attnT8, causal,
                        kq_act=False):
            """Per-head q/k DR projection software-pipelined with attention."""
            pqks = [None] * H
            for h in range(H):
                pqk = ppt("pqk")
                for qki, src in ((0, srcq_pairs), (1, srck_pairs)):
                    for i in range(NP):
                        nc.tensor.matmul(
                            pqk[0:HD, qki, :],
                            wqk_sb[:, qki, h, 2 * i : 2 * i + 2, :],
                            src[i],
                            start=(i == 0), stop=(i == NP - 1),
                            perf_mode=PM.DoubleRow,
                        )
                pqks[h] = pqk
                if h > 0:
                    attn_head_core(pqks[h - 1], bq, bk, h - 1, Vt, attnT8, causal, kq_act)
                    pqks[h - 1] = None
            attn_head_core(pqks[H - 1], bq, bk, H - 1, Vt, attnT8, causal, kq_act)

        def out_proj(attnT8, wo_sb, bias_sb, evict, pair_cb=None):
            """evict(e, po) consumes the [P, T] fp32 psum for block e.
            pair_cb(i) fires after blocks 2i, 2i+1 are evicted."""
            for e in range(ND):
                po = pst("po")
                for hp in range(H // 2):
                    nc.tensor.matmul(
                        po, wo_sb[:, e, 2 * hp : 2 * hp + 2, :],
                        attnT8[0:HD, 2 * hp : 2 * hp + 2, :],
                        start=(hp == 0), stop=(hp == H // 2 - 1),
                        perf_mode=PM.DoubleRow,
                    )
                evict(e, po)
                if pair_cb is not None and e % 2 == 1:
                    pair_cb(e // 2)

        def mlp_load(c):
            w1t = w1p.tile([P, FCC, ND, P], F8, name="w1t", tag="w1")
            nc.sync.dma_start(
                out=w1t,
                in_=mW1_d.rearrange("p (f a e) -> p f a e", f=NF, a=ND)[
                    :, c * FCC : (c + 1) * FCC, :, :
                ],
            )
            w2t = w2p.tile([P, FCC, D], BF, name="w2t", tag="w2")
            nc.sync.dma_start(
                out=w2t,
                in_=mW2_d.rearrange("p (f e) -> p f e", f=NF)[
                    :, c * FCC : (c + 1) * FCC, :
                ],
            )
            return w1t, w2t

        def mlp(xc3, x1T, outT, wpre):
            ph2 = [ppt(f"h2_{i}") for i in range(NP)]
            prev = None  # (fc, yt)
            NCHUNK = NF // FCC
            wpre.extend([mlp_load(0), mlp_load(1)])
            for c in range(NCHUNK):
                w1t, w2t = wpre[c]
                if c + 2 < NCHUNK:
                    wpre.append(mlp_load(c + 2))
                for j in range(FCC):
                    fc = c * FCC + j
                    ph1 = pst("ph1")
                    for i in range(NP):
                        nc.tensor.matmul(
                            ph1, w1t[:, j, 2 * i : 2 * i + 2, :], xc3[i],
                            start=(i == 0), stop=(i == NP - 1),
                            perf_mode=PM.DoubleRow,
                        )
                    yt = tmp.tile([P, T], BF, name="yt", tag="tmp")
                    nc.scalar.activation(
                        yt, ph1, AF.Gelu, bias=mB1_sb[:, fc : fc + 1], scale=RS
                    )
                    if prev is not None:
                        pfc, pyt, pw2, pj = prev
                        for e in range(ND):
                            nc.tensor.matmul(
                                ph2[e // 2][:, e % 2, :],
                                pw2[:, pj, e * P : (e + 1) * P], pyt,
                                start=(pfc == 0), stop=False,
                            )
                    prev = (fc, yt, w2t, j)
            pfc, pyt, pw2, pj = prev
            for e in range(ND):
                nc.tensor.matmul(
                    ph2[e // 2][:, e % 2, :], pw2[:, pj, e * P : (e + 1) * P], pyt,
                    start=False, stop=True,
                )
            for e in range(ND):
                nc.vector.scalar_tensor_tensor(
                    outT[e // 2][:, e % 2, :], ph2[e // 2][:, e % 2, :],
                    mB2_sb[:, e : e + 1], x1T[e // 2][:, e % 2, :],
                    ALU.add, ALU.add,
                )

        def _mark(phase):
            PHASE_MARKS.append((phase, int(nc.get_next_instruction_name()[2:])))

        def body():
            _mark("load_x")
            xT = []
            for i in range(NP):
                t = big.tile([P, 2, T], BF, name=f"xT{i}", tag=f"xT{i}", bufs=2)
                nc.sync.dma_start(
                    out=t,
                    in_=xT_d.rearrange("(a p) t -> p a t", p=P)[:, 2 * i : 2 * i + 2, :],
                )
                xT.append(t)
            # prefetch all attention weight stacks up-front (sync queue order)
            wv_sp = load_wv("sp")
            wqk_sp = load_wqk("sp")
            wo_sp = load_wo("sp")
            wv_tp = load_wv("tp")
            wqk_tp = load_wqk("tp")
            wo_tp = load_wo("tp")
            wv_cx = load_wv("cx")
            wqk_cx = load_wqk("cx")
            wo_cx = load_wo("cx")
            wpre = []

            _mark("ln0")
            xc = ln_all(xT, "xc")

            _mark("sp_v")
            Vt = big.tile([P, NT, H, HD + 2], F8, name="Vt_s", tag="Vt", bufs=2)
            gemm_v(xc, wv_sp, vbias["sp"], Vt)
            _mark("tp_v")
            Vt2 = big.tile([P, NT, H, HD + 2], F8, name="Vt_t", tag="Vt", bufs=2)
            gemm_v(xc, wv_tp, vbias["tp"], Vt2)

            _mark("sp_attn")
            attnT = big.tile([P, H, T], F8, name="attnT_s", tag="attnT", bufs=2)
            attn_branch(xc, xc, wqk_sp, bq96["sp"], bk96["sp"], Vt, attnT, False)
            _mark("sp_oproj")
            soT = [
                big.tile([P, 2, T], F8, name=f"soT{i}", tag=f"soT{i}", bufs=2)
                for i in range(NP)
            ]

            def evict_sp(e, po):
                nc.scalar.activation(
                    soT[e // 2][:, e % 2, :], po, AF.Identity,
                    bias=bo_sb["sp"][:, e : e + 1], scale=RS * RS,
                )

            out_proj(attnT, wo_sp, bo_sb["sp"], evict_sp)

            _mark("tp_attn")
            attnT2 = big.tile([P, H, T], F8, name="attnT_t", tag="attnT", bufs=2)
            attn_branch(xc, xc, wqk_tp, bq96["tp"], bk96["tp"], Vt2, attnT2, True,
                        kq_act=True)
            _mark("tp_oproj")
            toT = [
                big.tile([P, 2, T], F8, name=f"toT{i}", tag=f"toT{i}", bufs=2)
                for i in range(NP)
            ]

            def evict_tp(e, po):
                tb = tmp.tile([P, T], BF, name="tb", tag="tmp")
                nc.scalar.activation(
                    tb, po, AF.Identity,
                    bias=bo_sb["tp"][:, e : e + 1], scale=RS * RS,
                )
                # + xc * gamma_t (temporal residual, affine re-applied)
                nc.vector.scalar_tensor_tensor(
                    toT[e // 2][:, e % 2, :], xc[e // 2][:, e % 2, :],
                    gt_sb[:, e : e + 1], tb, ALU.mult, ALU.add,
                )

            out_proj(attnT2, wo_tp, bo_sb["tp"], evict_tp)

            _mark("cx_v")
            Vt3 = big.tile([P, NT, H, HD + 2], F8, name="Vt_c", tag="Vt", bufs=2)
            gemm_v(toT, wv_cx, vbias["cx"], Vt3)
            _mark("cx_attn")
            attnT3 = big.tile([P, H, T], F8, name="attnT_c", tag="attnT", bufs=2)
            attn_branch(soT, toT, wqk_cx, bq96["cx"], bk96["cx"], Vt3, attnT3, False)
            _mark("cx_oproj")
            x1T = [
                big.tile([P, 2, T], BF, name=f"x1T{i}", tag=f"x1T{i}", bufs=2)
                for i in range(NP)
            ]

            def evict_cx(e, po):
                tb = tmp.tile([P, T], BF, name="tb", tag="tmp")
                nc.scalar.activation(
                    tb, po, AF.Identity,
                    bias=bo_sb["cx"][:, e : e + 1], scale=RS * RS,
                )
                nc.vector.tensor_add(
                    x1T[e // 2][:, e % 2, :], tb, xT[e // 2][:, e % 2, :]
                )

            # ln3 stats stream into a psum pair as the cx out-projection
            # evicts each x1T block pair (attention's pp ring is done by now)
            pms3 = ppt("pms3")
            pm3 = pms3[:, 0, :]
            ps3 = pms3[:, 1, :]
            out_proj(attnT3, wo_cx, bo_sb["cx"], evict_cx,
                     pair_cb=lambda i: ln_feed(pm3, ps3, x1T, i))

            _mark("ln3")
            xc3 = ln_finish(pm3, ps3, x1T, "xc3")
            outT = [
                big.tile([P, 2, T], BF, name=f"outT{i}", tag=f"outT{i}", bufs=2)
                for i in range(NP)
            ]
            _mark("mlp")
            mlp(xc3, x1T, outT, wpre)
            for i in range(NP):
                nc.sync.dma_start(
                    out=outT_d.rearrange("(a p) t -> p a t", p=P)[
                        :, 2 * i : 2 * i + 2, :
                    ],
                    in_=outT[i],
                )

        if repeat == 1:
            body()
        else:
            with tc.For_i(0, repeat, 1):
                body()

    nc.compile()
    return nc


def _route(inputs):
    """Top-1 expert indices per sample, computed exactly as the reference
    (jax on CPU, f32) — softmax is monotonic so argmax of logits suffices."""
    import jax
    import jax.numpy as jnp

    cpu = jax.devices("cpu")[0]
    with jax.default_device(cpu):
        x = jnp.asarray(inputs["x"])
        h = jax.nn.gelu(
            x.mean(1) @ jnp.asarray(inputs["router_w1"]).T
            + jnp.asarray(inputs["router_b1"]),
            approximate=False,
        )
        logits = (
            h @ jnp.asarray(inputs["router_w2"]).T + jnp.asarray(inputs["router_b2"])
        )
        logits = np.asarray(logits)
    K = logits.shape[1] // 2
    idx_s = np.argmax(logits[:, :K], axis=-1)
    idx_t = np.argmax(logits[:, K:], axis=-1)
    return idx_s, idx_t


_cache = {}


def _get_nc(repeat=1):
    key = ("nc", repeat)
    if key not in _cache:
        _cache[key] = build(repeat=repeat)
    return _cache[key]


def _f(a):
    return np.ascontiguousarray(np.asarray(a), dtype=np.float32)


def _bf(a):
    return np.ascontiguousarray(np.asarray(a, dtype=np.float32).astype(NPBF))


def _f8(a):
    return np.ascontiguousarray(
        np.clip(np.asarray(a, dtype=np.float32) * WS, -240.0, 240.0).astype(NPF8)
    )


def _pack_qk_pair(wqT, wkT):
    # wqT/wkT: [D, D] = W^T columns (d, e); e = h*HD+k.
    arr = np.stack([np.asarray(wqT), np.asarray(wkT)])  # [2, D, D]
    arr = arr.reshape(2, ND, P, H, HD).transpose(2, 0, 3, 1, 4)
    return _f8(arr.reshape(P, 2 * H * ND * HD))


def _pack_v(wT):
    return _f8(np.asarray(wT).reshape(ND, P, D).transpose(1, 0, 2).reshape(P, ND * D))


def _pack_wo(w):
    wt = np.asarray(w).T.reshape(H, HD, ND, P)
    return _f8(wt.transpose(1, 2, 0, 3).reshape(HD, ND * H * P))


def _pack_w1(w1):
    w1t = np.asarray(w1).T.reshape(ND, P, NF, P)
    return _f8(w1t.transpose(1, 2, 0, 3).reshape(P, NF * ND * P))


def _pack_w2(w2):
    w2t = np.asarray(w2).T.reshape(NF, P, D)
    return _bf(w2t.transpose(1, 0, 2).reshape(P, NF * D))


def _branch_pack(m, pref, wqT, wkT, wvT, bq, bk, bv, wo, bo, g=None, be=None):
    """LN-affine fold: W^T rows scaled by gamma; beta folded into biases."""
    wqT = np.asarray(wqT, np.float64)
    wkT = np.asarray(wkT, np.float64)
    wvT = np.asarray(wvT, np.float64)
    if g is not None:
        gcol = np.asarray(g, np.float64)[:, None]
        bq = bq + np.asarray(be, np.float64) @ wqT
        bk = bk + np.asarray(be, np.float64) @ wkT
        bv = bv + np.asarray(be, np.float64) @ wvT
        wqT = wqT * gcol
        wkT = wkT * gcol
        wvT = wvT * gcol
    m[f"{pref}Wqk"] = _pack_qk_pair(wqT, wkT)
    m[f"{pref}Wv"] = _pack_v(wvT)
    m[f"{pref}Wo"] = _pack_wo(np.asarray(wo))
    m[f"{pref}Bq"] = _f(bq)
    m[f"{pref}Bk"] = _f(bk)
    m[f"{pref}Bv"] = _f(np.asarray(bv, np.float64) * WS)
    m[f"{pref}Bo"] = _f(bo)


def make_in_maps(inputs):
    idx_s, idx_t = _route(inputs)
    mask = np.tril(np.full((P, P), -1e9, dtype=np.float32), -1)

    g_s = _f(inputs["norm_s_g"])
    b_s = _f(inputs["norm_s_b"])
    g_t = _f(inputs["norm_t_g"])
    b_t = _f(inputs["norm_t_b"])
    g_m = _f(inputs["norm_mlp_g"])
    b_m = _f(inputs["norm_mlp_b"])

    cWqkvT = np.asarray(inputs["cross_wqkv"], np.float64).T
    cb = _f(inputs["cross_bqkv"])
    w1 = np.asarray(inputs["mlp_w1"], np.float64)
    w1T_eff = w1.T * np.asarray(g_m, np.float64)[:, None]
    b1_eff = _f(inputs["mlp_b1"]) + np.asarray(b_m, np.float64) @ w1.T

    shared = dict(
        mask=mask,
        g_t=g_t,
        mW1=_pack_w1(np.asarray(w1T_eff.T)),
        mB1=_f(b1_eff),
        mW2=_pack_w2(np.asarray(inputs["mlp_w2"])),
        mB2=_f(inputs["mlp_b2"]),
    )

    x = np.asarray(inputs["x"])
    in_maps = []
    for b in range(NCORES):
        s = int(idx_s[b])
        t = int(idx_t[b])
        m = dict(shared)
        m["xT"] = _bf(x[b].T)
        spWqkvT = np.asarray(inputs["sp_wqkv"])[s].astype(np.float64).T
        spb = _f(np.asarray(inputs["sp_bqkv"])[s])
        _branch_pack(
            m, "sp",
            spWqkvT[:, 0:D], spWqkvT[:, D : 2 * D], spWqkvT[:, 2 * D :],
            spb[0:D], spb[D : 2 * D], spb[2 * D :],
            np.asarray(inputs["sp_wo"])[s], _f(np.asarray(inputs["sp_bo"])[s]),
            g=g_s, be=b_s,
        )
        # temporal: bo_eff += beta_t (residual xn_t = xc*g_t + b_t; b_t into bias)
        _branch_pack(
            m, "tp",
            np.asarray(inputs["tp_wq"])[t].astype(np.float64).T,
            np.asarray(inputs["tp_wk"])[t].astype(np.float64).T,
            np.asarray(inputs["tp_wv"])[t].astype(np.float64).T,
            _f(np.asarray(inputs["tp_bq"])[t]), _f(np.asarray(inputs["tp_bk"])[t]),
            _f(np.asarray(inputs["tp_bv"])[t]),
            np.asarray(inputs["tp_wo"])[t],
            _f(np.asarray(inputs["tp_bo"])[t]) + b_t,
            g=g_t, be=b_t,
        )
        _branch_pack(
            m, "cx",
            cWqkvT[:, 0:D], cWqkvT[:, D : 2 * D], cWqkvT[:, 2 * D :],
            cb[0:D], cb[D : 2 * D], cb[2 * D :],
            np.asarray(inputs["cross_wo"]), _f(inputs["cross_bo"]),
        )
        in_maps.append(m)
    return in_maps


def kernel(**inputs) -> np.ndarray:
    repeat = int(os.environ.get("KREPEAT", "1"))
    nc = _get_nc(repeat=repeat)
    in_maps = make_in_maps(inputs)
    res = bass_utils.run_bass_kernel_spmd(nc, in_maps, core_ids=list(range(NCORES)))
    out = np.stack(
        [
            np.ascontiguousarray(
                np.asarray(res.results[b]["outT"], dtype=np.float32).T
            )
            for b in range(NCORES)
        ]
    )
    return out
